# revision 11
# baseline (speedup 1.0000x reference)
"""CrossAttention (single-head) Trainium2 kernel, 8-core data-parallel.

Full inputs in, full output out. Internally: batch 16 is sharded 2-per-core
across 8 NeuronCores; each core runs the whole attention layer for its two
batches in bf16 (f32 PSUM accumulation), with activations kept in transposed
[d, s] layout so every matmul contracts over the partition dim without any
on-chip transposes of large tensors.
"""

import sys

sys.path.insert(0, "/opt/trn_rl_repo")

import numpy as np
import ml_dtypes

import concourse.bass as bass
import concourse.mybir as mybir
import concourse.tile as tile
from concourse.bass_utils import run_bass_kernel_spmd

BF16 = mybir.dt.bfloat16
F32 = mybir.dt.float32
AF = mybir.ActivationFunctionType

N_CORES = 8
B, S, D = 16, 2048, 1024
NB = B // N_CORES          # batches per core
KC = D // 128              # 8 chunks of 128 along d
ST = S // 128              # 16 tiles of 128 along s
NBLK = S // 512            # 4 blocks of 512 along s
SCALE = 1.0 / np.sqrt(np.float32(D))  # 1/32


def _split_waits(nc, limit=1):
    """Walrus in this container allows at most one sync wait per instruction:
    hoist excess waits onto NoOp carriers inserted just before."""
    n_new = 0
    for f in nc.m.functions:
        for bb in f.blocks:
            new_insts = []
            for inst in bb.instructions:
                si = inst.sync_info
                waits = list(si.on_wait) if si and si.on_wait else []
                if len(waits) > limit:
                    excess, keep = waits[:-limit], waits[-limit:]
                    for i in range(0, len(excess), limit):
                        chunk = excess[i:i + limit]
                        nop = mybir.InstNoOp(
                            name=f"{inst.name}-ws-{n_new}",
                            ins=[], outs=[],
                            sync_info=mybir.SyncInfo(on_wait=chunk, on_update=[]),
                        )
                        nop.engine = inst.engine
                        new_insts.append(nop)
                        n_new += 1
                    si.on_wait = keep
                new_insts.append(inst)
            bb.instructions[:] = new_insts
    return n_new



def _strip_dead_pe_updates(nc):
    """Drop PE sem increments nobody waits on (Tile emits one per matmul;
    only group-stop indices are ever waited). Renumber wait thresholds by
    rank among kept updates — release timing is identical, PE saves ~26ns
    per dropped serialized EVT_SEM write. Straight-line programs only."""
    pe = mybir.EngineType.PE
    insts = [i for f in nc.m.functions for bb in f.blocks for i in bb.instructions]
    upd_by_sem, wait_by_sem, bad = {}, {}, set()
    for inst in insts:
        si = inst.sync_info
        if not si:
            continue
        for u in (si.on_update or []):
            if u.sync_type != "semaphore":
                continue
            if inst.engine != pe or u.update_mode != "sem-inc" or u.update_value != 1:
                bad.add(u.id)
            upd_by_sem.setdefault(u.id, []).append((inst, u))
        for w in (si.on_wait or []):
            if w.sync_type != "semaphore":
                continue
            if w.wait_mode != "sem-ge-imm" or w.wait_reg is not None:
                bad.add(w.id)
            wait_by_sem.setdefault(w.id, []).append(w)
    n_drop = 0
    for sem_id, ups in upd_by_sem.items():
        if sem_id in bad or sem_id not in wait_by_sem or len(ups) < 16:
            continue
        waited = sorted({w.wait_value for w in wait_by_sem[sem_id]})
        if not waited or waited[-1] > len(ups) or waited[0] < 1:
            continue
        keep = set(waited)
        rank = {t: k + 1 for k, t in enumerate(waited)}
        for idx, (inst, u) in enumerate(ups, start=1):
            if idx not in keep:
                inst.sync_info.on_update = [
                    x for x in inst.sync_info.on_update if x is not u
                ]
                n_drop += 1
        for w in wait_by_sem[sem_id]:
            w.wait_value = rank[w.wait_value]
    return n_drop


def _dedupe_ldweights(nc):
    """Drop InstLdweights whose weights AP equals the previous PE weight
    load with no intervening PE-array clobber (transpose or different
    load): the matmuls are non-self-loading (ldweights=False) so they
    reuse the already-loaded stationary operand. Sync carried by a
    dropped LDW transfers to the following instruction (Bacc later moves
    matmul waits back onto the nearest remaining LDW, which is merely
    more conservative). Per-block state so For_i bodies stay correct."""
    pe = mybir.EngineType.PE
    n_drop = 0
    for f in nc.m.functions:
        for bb in f.blocks:
            insts = bb.instructions
            keep = []
            last_sig = None
            pend_waits, pend_ups = [], []
            for inst in insts:
                tn = type(inst).__name__
                eng = getattr(inst, "engine", None)
                if tn == "InstLdweights":
                    ap = inst.ins[0]
                    sig = (ap.memref, ap.offset, str(ap.ap), str(ap.dtype))
                    if sig == last_sig:
                        si = inst.sync_info
                        if si:
                            pend_waits += list(si.on_wait or [])
                            pend_ups += list(si.on_update or [])
                        n_drop += 1
                        continue
                    last_sig = sig
                elif eng == pe:
                    if tn == "InstMatmult":
                        if inst.is_transpose:
                            last_sig = None
                    elif tn not in ("InstNoOp", "InstEventSemaphore", "InstDrain"):
                        last_sig = None
                if (pend_waits or pend_ups) and eng == pe:
                    si = inst.sync_info
                    if si is None:
                        inst.sync_info = mybir.SyncInfo(
                            on_wait=pend_waits, on_update=pend_ups
                        )
                    else:
                        si.on_wait = pend_waits + list(si.on_wait or [])
                        si.on_update = list(si.on_update or []) + pend_ups
                    pend_waits, pend_ups = [], []
                keep.append(inst)
            assert not pend_waits and not pend_ups, "dangling LDW sync at block end"
            insts[:] = keep
    return n_drop


def build_program(reps=1):
    """reps>1 wraps the whole computation in a hardware For_i loop — used
    only for timing (slope over reps isolates on-silicon exec time from
    per-call NEFF load overhead)."""
    nc = bass.Bass()

    qT_d = nc.declare_dram_parameter("qT", [NB, D, S], BF16, isOutput=False)
    kT_d = nc.declare_dram_parameter("kT", [NB, D, S], BF16, isOutput=False)
    vT_d = nc.declare_dram_parameter("vT", [NB, D, S], BF16, isOutput=False)
    Wq_d = nc.declare_dram_parameter("Wq", [D, D], BF16, isOutput=False)
    Wk_d = nc.declare_dram_parameter("Wk", [D, D], BF16, isOutput=False)
    Wv_d = nc.declare_dram_parameter("Wv", [D, D], BF16, isOutput=False)
    Wo_d = nc.declare_dram_parameter("Wo", [D, D], BF16, isOutput=False)
    # bq pre-scaled by 1/32 and reshaped [128, KC] host-side; bk likewise unscaled
    bq_d = nc.declare_dram_parameter("bq", [128, KC], F32, isOutput=False)
    bk_d = nc.declare_dram_parameter("bk", [128, KC], F32, isOutput=False)
    bv_d = nc.declare_dram_parameter("bv", [D], BF16, isOutput=False)
    bo_d = nc.declare_dram_parameter("bo", [D], BF16, isOutput=False)
    out_d = nc.declare_dram_parameter("out", [NB, S, D], F32, isOutput=True)

    from contextlib import ExitStack
    with tile.TileContext(nc) as tc:
        with ExitStack() as _stk:
            _p = lambda **kw: _stk.enter_context(tc.tile_pool(**kw))
            wqopool = _p(name="wqo", bufs=8)
            wkvpool = _p(name="wkv", bufs=9)
            inpool = _p(name="inp", bufs=16)
            kpool = _p(name="keyT", bufs=8)
            vpool = _p(name="value", bufs=1)
            qpool = _p(name="queryT", bufs=12)
            epool = _p(name="expT", bufs=2)
            upool = _p(name="UT", bufs=2)
            opool = _p(name="outb", bufs=2)
            sumpool = _p(name="sums", bufs=2)
            rpool = _p(name="rpool", bufs=2)
            cpool = _p(name="const", bufs=1)
            pspool = _p(name="ps", bufs=5, space="PSUM")
            ps1pool = _p(name="ps1", bufs=1, space="PSUM")
            psrpool = _p(name="psr", bufs=2, space="PSUM")
            # constants
            ones = cpool.tile([128, 1], BF16, tag="ones")
            nc.vector.memset(ones[:], 1.0)
            ident = cpool.tile([1, 1], F32, tag="ident")
            nc.vector.memset(ident[:], 1.0)
            bq_sb = cpool.tile([128, KC], F32, tag="bq")
            nc.sync.dma_start(out=bq_sb[:], in_=bq_d[:])
            bk_sb = cpool.tile([128, KC], F32, tag="bk")
            nc.sync.dma_start(out=bk_sb[:], in_=bk_d[:])
            bv_sb = cpool.tile([128, D], BF16, tag="bv")
            ap = bv_d[:]
            nc.sync.dma_start(
                out=bv_sb[:],
                in_=bass.AP(tensor=ap.tensor, offset=ap.offset, ap=[[0, 128]] + ap.ap),
            )
            bo_sb = cpool.tile([128, D], BF16, tag="bo")
            ap = bo_d[:]
            nc.sync.dma_start(
                out=bo_sb[:],
                in_=bass.AP(tensor=ap.tensor, offset=ap.offset, ap=[[0, 128]] + ap.ap),
            )

            def load_w(w_d, pool, tag):
                tiles = []
                for i in range(KC):
                    t = pool.tile([128, D], BF16, tag=tag, name=f"{tag}{i}")
                    nc.sync.dma_start(out=t[:], in_=w_d[i * 128:(i + 1) * 128, :])
                    tiles.append(t)
                return tiles

            # critical-path first: Wk and the first kin block feed the very
            # first matmuls — queue them ahead of the resident Wq/Wo loads.
            # Only for the straight-line (reps==1) program: inside a For_i
            # the hoisted tiles' ring slots get recycled across iterations.
            hoist = reps == 1
            if hoist:
                Wk_first = load_w(Wk_d, wkvpool, "wkv")
                kin_first = []
                for i in range(KC):
                    t = inpool.tile([128, 512], BF16, tag="inp", name=f"in{i}")
                    nc.sync.dma_start(out=t[:], in_=kT_d[0, i * 128:(i + 1) * 128, 0:512])
                    kin_first.append(t)
            # Wq and Wo stay resident for the whole kernel
            Wq_t = load_w(Wq_d, wqopool, "wq")
            Wo_t = load_w(Wo_d, wqopool, "wo")

            import contextlib
            loop_ctx = tc.For_i(0, reps, 1) if reps > 1 else contextlib.nullcontext()
            with loop_ctx:
              for b in range(NB):
                  # ---------------- keyT[d, s] = Wk.T @ kT (+bk) ----------------
                  Wk_t = Wk_first if (hoist and b == 0) else load_w(Wk_d, wkvpool, "wkv")
                  keyT = [kpool.tile([128, S], BF16, tag="keyT", name=f"keyT{i}") for i in range(KC)]
                  # s-block pairs: the stationary Wk chunk is shared by the
                  # two interleaved psum groups (deduped LDW)
                  for sp in range(NBLK // 2):
                      kin2 = []
                      for h in range(2):
                          s = sp * 2 + h
                          if hoist and b == 0 and s == 0:
                              kin2.append(kin_first)
                              continue
                          kin = []
                          for i in range(KC):
                              t = inpool.tile([128, 512], BF16, tag="inp", name=f"in{i}")
                              nc.sync.dma_start(
                                  out=t[:],
                                  in_=kT_d[b, i * 128:(i + 1) * 128, s * 512:(s + 1) * 512],
                              )
                              kin.append(t)
                          kin2.append(kin)
                      for do in range(KC):
                          psn = [pspool.tile([128, 512], F32, tag="ps", name=f"ps{h}")
                                 for h in range(2)]
                          for i in range(KC):
                              for h in range(2):
                                  nc.tensor.matmul(
                                      psn[h][:], Wk_t[i][:, do * 128:(do + 1) * 128],
                                      kin2[h][i][:],
                                      start=(i == 0), stop=(i == KC - 1),
                                  )
                          for h in range(2):
                              s = sp * 2 + h
                              nc.vector.tensor_scalar_add(
                                  keyT[do][:, s * 512:(s + 1) * 512], psn[h][:],
                                  bk_sb[:, do:do + 1],
                              )

                  # ---------------- value[s, d] = vT.T @ Wv (+bv) ----------------
                  Wv_t = load_w(Wv_d, wkvpool, "wkv")
                  val = vpool.tile([128, ST, D], BF16, tag="value")
                  for s in range(NBLK):
                      vin = []
                      for i in range(KC):
                          t = inpool.tile([128, 512], BF16, tag="inp", name=f"in{i}")
                          nc.sync.dma_start(
                              out=t[:],
                              in_=vT_d[b, i * 128:(i + 1) * 128, s * 512:(s + 1) * 512],
                          )
                          vin.append(t)
                      for tt in range(4):
                          t16 = s * 4 + tt
                          # n-pair shares the stationary vin chunk: the two
                          # psum groups interleave so consecutive matmuls
                          # reuse the loaded weights (deduped LDW)
                          psn = [pspool.tile([128, 512], F32, tag="ps", name=f"ps{n}")
                                 for n in range(2)]
                          for i in range(KC):
                              for n in range(2):
                                  nc.tensor.matmul(
                                      psn[n][:],
                                      vin[i][:, tt * 128:(tt + 1) * 128],
                                      Wv_t[i][:, n * 512:(n + 1) * 512],
                                      start=(i == 0), stop=(i == KC - 1),
                                  )
                          for n in range(2):
                              nc.vector.tensor_add(
                                  val[:, t16, n * 512:(n + 1) * 512], psn[n][:],
                                  bv_sb[:, n * 512:(n + 1) * 512],
                              )

                  # ---------------- per 512-wide sq block ----------------
                  for blk in range(NBLK):
                      # queryT block [d, 512] = Wq.T @ qT_blk, scaled 1/32 (+bq/32)
                      qin = []
                      for i in range(KC):
                          t = inpool.tile([128, 512], BF16, tag="inp", name=f"in{i}")
                          nc.sync.dma_start(
                              out=t[:],
                              in_=qT_d[b, i * 128:(i + 1) * 128, blk * 512:(blk + 1) * 512],
                          )
                          qin.append(t)
                      qry = []
                      for do in range(KC):
                          psum = pspool.tile([128, 512], F32, tag="ps")
                          for i in range(KC):
                              nc.tensor.matmul(
                                  psum[:], Wq_t[i][:, do * 128:(do + 1) * 128], qin[i][:],
                                  start=(i == 0), stop=(i == KC - 1),
                              )
                          qt = qpool.tile([128, 512], BF16, tag="queryT", name=f"qry{do}")
                          nc.vector.tensor_scalar(
                              out=qt[:], in0=psum[:], scalar1=float(SCALE),
                              scalar2=bq_sb[:, do:do + 1],
                              op0=mybir.AluOpType.mult, op1=mybir.AluOpType.add,
                          )
                          qry.append(qt)

                      # scoresT -> expT
                      exp_blk = epool.tile([128, ST, 512], BF16, tag="expT")
                      for t16 in range(ST):
                          psum = pspool.tile([128, 512], F32, tag="ps")
                          for i in range(KC):
                              nc.tensor.matmul(
                                  psum[:],
                                  keyT[i][:, t16 * 128:(t16 + 1) * 128],
                                  qry[i][:],
                                  start=(i == 0), stop=(i == KC - 1),
                              )
                          nc.scalar.activation(exp_blk[:, t16, :], psum[:], AF.Exp)

                      # column sums over all sk (partition dim) via ones-matmul
                      sums_ps = ps1pool.tile([1, 512], F32, tag="ps1")
                      for t16 in range(ST):
                          nc.tensor.matmul(
                              sums_ps[:], ones[:], exp_blk[:, t16, :],
                              start=(t16 == 0), stop=(t16 == ST - 1),
                          )
                      sums_sb = sumpool.tile([1, 512], F32, tag="sums")
                      nc.vector.tensor_copy(sums_sb[:], sums_ps[:])

                      # r = 1/sums as per-partition scalars, via [1,128] PE
                      # transpose; emitted before UT so its PE<->DVE chain is
                      # hidden under the UT matmul stream
                      r_sb = rpool.tile([128, 4], F32, tag="r")
                      for m in range(4):
                          pr = psrpool.tile([128, 1], F32, tag="psr")
                          nc.tensor.transpose(
                              pr[:], sums_sb[0:1, m * 128:(m + 1) * 128], ident[:]
                          )
                          nc.vector.reciprocal(r_sb[:, m:m + 1], pr[:])

                      # UT block [d, 512] = value.T @ expT
                      ut = upool.tile([128, KC, 512], BF16, tag="UT")
                      for j in range(KC):
                          psum = pspool.tile([128, 512], F32, tag="ps")
                          for t16 in range(ST):
                              nc.tensor.matmul(
                                  psum[:],
                                  val[:, t16, j * 128:(j + 1) * 128],
                                  exp_blk[:, t16, :],
                                  start=(t16 == 0), stop=(t16 == ST - 1),
                              )
                          nc.vector.tensor_copy(ut[:, j, :], psum[:])

                      # final block: out[sq, d] = (UT.T @ Wo) * r + bo
                      # n-pair shares the stationary ut chunk (deduped LDW)
                      for m in range(4):
                          ob = opool.tile([128, D], F32, tag="outb")
                          psn = [pspool.tile([128, 512], F32, tag="ps", name=f"ps{n}")
                                 for n in range(2)]
                          for j in range(KC):
                              for n in range(2):
                                  nc.tensor.matmul(
                                      psn[n][:],
                                      ut[:, j, m * 128:(m + 1) * 128],
                                      Wo_t[j][:, n * 512:(n + 1) * 512],
                                      start=(j == 0), stop=(j == KC - 1),
                                  )
                          for n in range(2):
                              nc.vector.tensor_scalar_mul(
                                  ob[:, n * 512:(n + 1) * 512], psn[n][:], r_sb[:, m:m + 1]
                              )
                              nc.vector.tensor_add(
                                  ob[:, n * 512:(n + 1) * 512],
                                  ob[:, n * 512:(n + 1) * 512],
                                  bo_sb[:, n * 512:(n + 1) * 512],
                              )
                          sq = blk * 512 + m * 128
                          nc.sync.dma_start(out=out_d[b, sq:sq + 128, :], in_=ob[:])

    n_ldw = _dedupe_ldweights(nc)
    if reps == 1:
        _strip_dead_pe_updates(nc)
        print(f"deduped {n_ldw} ldweights", file=sys.stderr)
    _split_waits(nc)
    return nc


_PROGRAM = None


def _get_program():
    global _PROGRAM
    if _PROGRAM is None:
        _PROGRAM = build_program()
    return _PROGRAM


def prepare_in_maps(q, k, v, Wq, bq, Wk, bk, Wv, bv, Wo, bo):
    bf = ml_dtypes.bfloat16
    f32 = np.float32

    def t_bf16(x):  # [B,S,D] f32 -> [B,D,S] bf16 contiguous
        return np.ascontiguousarray(
            np.asarray(x, f32).astype(bf).transpose(0, 2, 1)
        )

    qT = t_bf16(q)
    kT = t_bf16(k)
    vT = t_bf16(v)
    Wq_b = np.asarray(Wq, f32).astype(bf)
    Wk_b = np.asarray(Wk, f32).astype(bf)
    Wv_b = np.asarray(Wv, f32).astype(bf)
    Wo_b = np.asarray(Wo, f32).astype(bf)
    bq2 = np.ascontiguousarray(
        (np.asarray(bq, f32) * np.float32(SCALE)).reshape(KC, 128).T
    )
    bk2 = np.ascontiguousarray(np.asarray(bk, f32).reshape(KC, 128).T)
    bv1 = np.ascontiguousarray(np.asarray(bv, f32)).astype(bf)
    bo1 = np.ascontiguousarray(np.asarray(bo, f32)).astype(bf)

    in_maps = []
    for c in range(N_CORES):
        sl = slice(c * NB, (c + 1) * NB)
        in_maps.append({
            "qT": qT[sl], "kT": kT[sl], "vT": vT[sl],
            "Wq": Wq_b, "Wk": Wk_b, "Wv": Wv_b, "Wo": Wo_b,
            "bq": bq2, "bk": bk2, "bv": bv1, "bo": bo1,
        })
    return in_maps


def kernel(q, k, v, Wq, bq, Wk, bk, Wv, bv, Wo, bo):
    nc = _get_program()
    in_maps = prepare_in_maps(q, k, v, Wq, bq, Wk, bk, Wv, bv, Wo, bo)
    res = run_bass_kernel_spmd(nc, in_maps, core_ids=list(range(N_CORES)))
    out = np.concatenate([res.results[c]["out"] for c in range(N_CORES)], axis=0)
    return out.astype(np.float32)



# revision 16
# speedup vs baseline: 1.0617x; 1.0617x over previous
"""CrossAttention (single-head) Trainium2 kernel, 8-core data-parallel.

Full inputs in, full output out. Internally: batch 16 is sharded 2-per-core
across 8 NeuronCores; each core runs the whole attention layer for its two
batches in bf16 (f32 PSUM accumulation), with activations kept in transposed
[d, s] layout so every matmul contracts over the partition dim without any
on-chip transposes of large tensors.
"""

import sys

sys.path.insert(0, "/opt/trn_rl_repo")

import numpy as np
import ml_dtypes

import concourse.bass as bass
import concourse.mybir as mybir
import concourse.tile as tile
from concourse.bass_utils import run_bass_kernel_spmd

BF16 = mybir.dt.bfloat16
F32 = mybir.dt.float32
AF = mybir.ActivationFunctionType

N_CORES = 8
B, S, D = 16, 2048, 1024
NB = B // N_CORES          # batches per core
KC = D // 128              # 8 chunks of 128 along d
ST = S // 128              # 16 tiles of 128 along s
NBLK = S // 512            # 4 blocks of 512 along s
SCALE = 1.0 / np.sqrt(np.float32(D))  # 1/32


def _split_waits(nc, limit=1):
    """Walrus in this container allows at most one sync wait per instruction:
    hoist excess waits onto NoOp carriers inserted just before."""
    n_new = 0
    for f in nc.m.functions:
        for bb in f.blocks:
            new_insts = []
            for inst in bb.instructions:
                si = inst.sync_info
                waits = list(si.on_wait) if si and si.on_wait else []
                if len(waits) > limit:
                    excess, keep = waits[:-limit], waits[-limit:]
                    for i in range(0, len(excess), limit):
                        chunk = excess[i:i + limit]
                        nop = mybir.InstNoOp(
                            name=f"{inst.name}-ws-{n_new}",
                            ins=[], outs=[],
                            sync_info=mybir.SyncInfo(on_wait=chunk, on_update=[]),
                        )
                        nop.engine = inst.engine
                        new_insts.append(nop)
                        n_new += 1
                    si.on_wait = keep
                new_insts.append(inst)
            bb.instructions[:] = new_insts
    return n_new



def _strip_dead_pe_updates(nc):
    """Drop PE sem increments nobody waits on (Tile emits one per matmul;
    only group-stop indices are ever waited). Renumber wait thresholds by
    rank among kept updates — release timing is identical, PE saves ~26ns
    per dropped serialized EVT_SEM write. Straight-line programs only."""
    pe = mybir.EngineType.PE
    insts = [i for f in nc.m.functions for bb in f.blocks for i in bb.instructions]
    upd_by_sem, wait_by_sem, bad = {}, {}, set()
    for inst in insts:
        si = inst.sync_info
        if not si:
            continue
        for u in (si.on_update or []):
            if u.sync_type != "semaphore":
                continue
            if inst.engine != pe or u.update_mode != "sem-inc" or u.update_value != 1:
                bad.add(u.id)
            upd_by_sem.setdefault(u.id, []).append((inst, u))
        for w in (si.on_wait or []):
            if w.sync_type != "semaphore":
                continue
            if w.wait_mode != "sem-ge-imm" or w.wait_reg is not None:
                bad.add(w.id)
            wait_by_sem.setdefault(w.id, []).append(w)
    n_drop = 0
    for sem_id, ups in upd_by_sem.items():
        if sem_id in bad or sem_id not in wait_by_sem or len(ups) < 16:
            continue
        waited = sorted({w.wait_value for w in wait_by_sem[sem_id]})
        if not waited or waited[-1] > len(ups) or waited[0] < 1:
            continue
        keep = set(waited)
        rank = {t: k + 1 for k, t in enumerate(waited)}
        for idx, (inst, u) in enumerate(ups, start=1):
            if idx not in keep:
                inst.sync_info.on_update = [
                    x for x in inst.sync_info.on_update if x is not u
                ]
                n_drop += 1
        for w in wait_by_sem[sem_id]:
            w.wait_value = rank[w.wait_value]
    return n_drop


def _dedupe_ldweights(nc):
    """Drop InstLdweights whose weights AP equals the previous PE weight
    load with no intervening PE-array clobber (transpose or different
    load): the matmuls are non-self-loading (ldweights=False) so they
    reuse the already-loaded stationary operand. Sync carried by a
    dropped LDW transfers to the next PE instruction (Bacc later moves
    matmul waits onto the nearest remaining LDW, which is merely more
    conservative). Per-block state so For_i bodies stay correct."""
    pe = mybir.EngineType.PE
    n_drop = 0
    for f in nc.m.functions:
        for bb in f.blocks:
            insts = bb.instructions
            keep = []
            last_sig = None
            pend_waits, pend_ups = [], []
            for inst in insts:
                tn = type(inst).__name__
                eng = getattr(inst, "engine", None)
                if tn == "InstLdweights":
                    ap = inst.ins[0]
                    sig = (ap.memref, ap.offset, str(ap.ap), str(ap.dtype))
                    if sig == last_sig:
                        si = inst.sync_info
                        if si:
                            pend_waits += list(si.on_wait or [])
                            pend_ups += list(si.on_update or [])
                        n_drop += 1
                        continue
                    last_sig = sig
                elif eng == pe:
                    if tn == "InstMatmult":
                        if inst.is_transpose:
                            last_sig = None
                    elif tn not in ("InstNoOp", "InstEventSemaphore", "InstDrain"):
                        last_sig = None
                if (pend_waits or pend_ups) and eng == pe:
                    si = inst.sync_info
                    if si is None:
                        inst.sync_info = mybir.SyncInfo(
                            on_wait=pend_waits, on_update=pend_ups
                        )
                    else:
                        si.on_wait = pend_waits + list(si.on_wait or [])
                        si.on_update = list(si.on_update or []) + pend_ups
                    pend_waits, pend_ups = [], []
                keep.append(inst)
            assert not pend_waits and not pend_ups, "dangling LDW sync at block end"
            insts[:] = keep
    return n_drop


def build_program(reps=1):
    """reps>1 wraps the whole computation in a hardware For_i loop — used
    only for timing (slope over reps isolates on-silicon exec time from
    per-call NEFF load overhead)."""
    nc = bass.Bass()

    qT_d = nc.declare_dram_parameter("qT", [NB, D, S], BF16, isOutput=False)
    kT_d = nc.declare_dram_parameter("kT", [NB, D, S], BF16, isOutput=False)
    vT_d = nc.declare_dram_parameter("vT", [NB, D, S], BF16, isOutput=False)
    Wq_d = nc.declare_dram_parameter("Wq", [D, D], BF16, isOutput=False)
    Wk_d = nc.declare_dram_parameter("Wk", [D, D], BF16, isOutput=False)
    Wv_d = nc.declare_dram_parameter("Wv", [D, D], BF16, isOutput=False)
    Wo_d = nc.declare_dram_parameter("Wo", [D, D], BF16, isOutput=False)
    # bq pre-scaled by 1/32 and reshaped [128, KC] host-side; bk likewise unscaled
    bq_d = nc.declare_dram_parameter("bq", [128, KC], F32, isOutput=False)
    bk_d = nc.declare_dram_parameter("bk", [128, KC], F32, isOutput=False)
    bv_d = nc.declare_dram_parameter("bv", [D], BF16, isOutput=False)
    bo_d = nc.declare_dram_parameter("bo", [D], BF16, isOutput=False)
    out_d = nc.declare_dram_parameter("out", [NB, S, D], F32, isOutput=True)

    from contextlib import ExitStack
    with tile.TileContext(nc) as tc:
        with ExitStack() as _stk:
            _p = lambda **kw: _stk.enter_context(tc.tile_pool(**kw))
            wqopool = _p(name="wqo", bufs=8)
            wkvpool = _p(name="wkv", bufs=9)
            inpool = _p(name="inp", bufs=16)
            kpool = _p(name="keyT", bufs=8)
            vpool = _p(name="value", bufs=1)
            qpool = _p(name="queryT", bufs=12)
            epool = _p(name="expT", bufs=2)
            upool = _p(name="UT", bufs=2)
            opool = _p(name="outb", bufs=2)
            sumpool = _p(name="sums", bufs=2)
            rpool = _p(name="rpool", bufs=2)
            cpool = _p(name="const", bufs=1)
            pspool = _p(name="ps", bufs=5, space="PSUM")
            ps1pool = _p(name="ps1", bufs=1, space="PSUM")
            psrpool = _p(name="psr", bufs=2, space="PSUM")
            # constants
            ones = cpool.tile([128, 1], BF16, tag="ones")
            nc.vector.memset(ones[:], 1.0)
            ident = cpool.tile([1, 1], F32, tag="ident")
            nc.vector.memset(ident[:], 1.0)
            bq_sb = cpool.tile([128, KC], F32, tag="bq")
            nc.sync.dma_start(out=bq_sb[:], in_=bq_d[:])
            bk_sb = cpool.tile([128, KC], F32, tag="bk")
            nc.sync.dma_start(out=bk_sb[:], in_=bk_d[:])
            bv_sb = cpool.tile([128, D], BF16, tag="bv")
            ap = bv_d[:]
            nc.sync.dma_start(
                out=bv_sb[:],
                in_=bass.AP(tensor=ap.tensor, offset=ap.offset, ap=[[0, 128]] + ap.ap),
            )
            bo_sb = cpool.tile([128, D], BF16, tag="bo")
            ap = bo_d[:]
            nc.sync.dma_start(
                out=bo_sb[:],
                in_=bass.AP(tensor=ap.tensor, offset=ap.offset, ap=[[0, 128]] + ap.ap),
            )

            def load_w(w_d, pool, tag):
                tiles = []
                for i in range(KC):
                    t = pool.tile([128, D], BF16, tag=tag, name=f"{tag}{i}")
                    nc.sync.dma_start(out=t[:], in_=w_d[i * 128:(i + 1) * 128, :])
                    tiles.append(t)
                return tiles

            # critical-path first: Wk and the first kin block feed the very
            # first matmuls — queue them ahead of the resident Wq/Wo loads.
            # Only for the straight-line (reps==1) program: inside a For_i
            # the hoisted tiles' ring slots get recycled across iterations.
            hoist = reps == 1
            if hoist:
                Wk_first = load_w(Wk_d, wkvpool, "wkv")
                kin_first = []
                for i in range(KC):
                    t = inpool.tile([128, 512], BF16, tag="inp", name=f"in{i}")
                    nc.sync.dma_start(out=t[:], in_=kT_d[0, i * 128:(i + 1) * 128, 0:512])
                    kin_first.append(t)
            # Wq and Wo stay resident for the whole kernel
            Wq_t = load_w(Wq_d, wqopool, "wq")
            Wo_t = load_w(Wo_d, wqopool, "wo")

            import contextlib
            loop_ctx = tc.For_i(0, reps, 1) if reps > 1 else contextlib.nullcontext()
            with loop_ctx:
              for b in range(NB):
                  # ---------------- keyT[d, s] = Wk.T @ kT (+bk) ----------------
                  Wk_t = Wk_first if (hoist and b == 0) else load_w(Wk_d, wkvpool, "wkv")
                  keyT = [kpool.tile([128, S], BF16, tag="keyT", name=f"keyT{i}") for i in range(KC)]
                  for s in range(NBLK):
                      if hoist and b == 0 and s == 0:
                          kin = kin_first
                      else:
                          kin = []
                          for i in range(KC):
                              t = inpool.tile([128, 512], BF16, tag="inp", name=f"in{i}")
                              nc.sync.dma_start(
                                  out=t[:],
                                  in_=kT_d[b, i * 128:(i + 1) * 128, s * 512:(s + 1) * 512],
                              )
                              kin.append(t)
                      for do in range(KC):
                          psum = pspool.tile([128, 512], F32, tag="ps")
                          for i in range(KC):
                              nc.tensor.matmul(
                                  psum[:], Wk_t[i][:, do * 128:(do + 1) * 128], kin[i][:],
                                  start=(i == 0), stop=(i == KC - 1),
                              )
                          nc.vector.tensor_scalar_add(
                              keyT[do][:, s * 512:(s + 1) * 512], psum[:],
                              bk_sb[:, do:do + 1],
                          )

                  # ---------------- value[s, d] = vT.T @ Wv (+bv) ----------------
                  Wv_t = load_w(Wv_d, wkvpool, "wkv")
                  val = vpool.tile([128, ST, D], BF16, tag="value")
                  for s in range(NBLK):
                      vin = []
                      for i in range(KC):
                          t = inpool.tile([128, 512], BF16, tag="inp", name=f"in{i}")
                          nc.sync.dma_start(
                              out=t[:],
                              in_=vT_d[b, i * 128:(i + 1) * 128, s * 512:(s + 1) * 512],
                          )
                          vin.append(t)
                      for tt in range(4):
                          t16 = s * 4 + tt
                          for n in range(2):
                              psum = pspool.tile([128, 512], F32, tag="ps")
                              for i in range(KC):
                                  nc.tensor.matmul(
                                      psum[:],
                                      vin[i][:, tt * 128:(tt + 1) * 128],
                                      Wv_t[i][:, n * 512:(n + 1) * 512],
                                      start=(i == 0), stop=(i == KC - 1),
                                  )
                              nc.vector.tensor_add(
                                  val[:, t16, n * 512:(n + 1) * 512], psum[:],
                                  bv_sb[:, n * 512:(n + 1) * 512],
                              )

                  # ---------------- per 512-wide sq block ----------------
                  for blk in range(NBLK):
                      # queryT block [d, 512] = Wq.T @ qT_blk, scaled 1/32 (+bq/32)
                      qin = []
                      for i in range(KC):
                          t = inpool.tile([128, 512], BF16, tag="inp", name=f"in{i}")
                          nc.sync.dma_start(
                              out=t[:],
                              in_=qT_d[b, i * 128:(i + 1) * 128, blk * 512:(blk + 1) * 512],
                          )
                          qin.append(t)
                      qry = []
                      for do in range(KC):
                          psum = pspool.tile([128, 512], F32, tag="ps")
                          for i in range(KC):
                              nc.tensor.matmul(
                                  psum[:], Wq_t[i][:, do * 128:(do + 1) * 128], qin[i][:],
                                  start=(i == 0), stop=(i == KC - 1),
                              )
                          qt = qpool.tile([128, 512], BF16, tag="queryT", name=f"qry{do}")
                          nc.vector.tensor_scalar(
                              out=qt[:], in0=psum[:], scalar1=float(SCALE),
                              scalar2=bq_sb[:, do:do + 1],
                              op0=mybir.AluOpType.mult, op1=mybir.AluOpType.add,
                          )
                          qry.append(qt)

                      # scoresT -> expT
                      exp_blk = epool.tile([128, ST, 512], BF16, tag="expT")
                      for t16 in range(ST):
                          psum = pspool.tile([128, 512], F32, tag="ps")
                          for i in range(KC):
                              nc.tensor.matmul(
                                  psum[:],
                                  keyT[i][:, t16 * 128:(t16 + 1) * 128],
                                  qry[i][:],
                                  start=(i == 0), stop=(i == KC - 1),
                              )
                          nc.scalar.activation(exp_blk[:, t16, :], psum[:], AF.Exp)

                      # column sums over all sk (partition dim) via ones-matmul
                      sums_ps = ps1pool.tile([1, 512], F32, tag="ps1")
                      for t16 in range(ST):
                          nc.tensor.matmul(
                              sums_ps[:], ones[:], exp_blk[:, t16, :],
                              start=(t16 == 0), stop=(t16 == ST - 1),
                          )
                      sums_sb = sumpool.tile([1, 512], F32, tag="sums")
                      nc.vector.tensor_copy(sums_sb[:], sums_ps[:])

                      # r = 1/sums as per-partition scalars, via [1,128] PE
                      # transpose; emitted before UT so its PE<->DVE chain is
                      # hidden under the UT matmul stream
                      r_sb = rpool.tile([128, 4], F32, tag="r")
                      for m in range(4):
                          pr = psrpool.tile([128, 1], F32, tag="psr")
                          nc.tensor.transpose(
                              pr[:], sums_sb[0:1, m * 128:(m + 1) * 128], ident[:]
                          )
                          nc.vector.reciprocal(r_sb[:, m:m + 1], pr[:])

                      # UT block [d, 512] = value.T @ expT
                      ut = upool.tile([128, KC, 512], BF16, tag="UT")
                      for j in range(KC):
                          psum = pspool.tile([128, 512], F32, tag="ps")
                          for t16 in range(ST):
                              nc.tensor.matmul(
                                  psum[:],
                                  val[:, t16, j * 128:(j + 1) * 128],
                                  exp_blk[:, t16, :],
                                  start=(t16 == 0), stop=(t16 == ST - 1),
                              )
                          nc.vector.tensor_copy(ut[:, j, :], psum[:])

                      # final block: out[sq, d] = (UT.T @ Wo) * r + bo
                      for m in range(4):
                          ob = opool.tile([128, D], F32, tag="outb")
                          for n in range(2):
                              psum = pspool.tile([128, 512], F32, tag="ps")
                              for j in range(KC):
                                  nc.tensor.matmul(
                                      psum[:],
                                      ut[:, j, m * 128:(m + 1) * 128],
                                      Wo_t[j][:, n * 512:(n + 1) * 512],
                                      start=(j == 0), stop=(j == KC - 1),
                                  )
                              nc.vector.tensor_scalar_mul(
                                  ob[:, n * 512:(n + 1) * 512], psum[:], r_sb[:, m:m + 1]
                              )
                              nc.vector.tensor_add(
                                  ob[:, n * 512:(n + 1) * 512],
                                  ob[:, n * 512:(n + 1) * 512],
                                  bo_sb[:, n * 512:(n + 1) * 512],
                              )
                          sq = blk * 512 + m * 128
                          nc.sync.dma_start(out=out_d[b, sq:sq + 128, :], in_=ob[:])

    _dedupe_ldweights(nc)
    # Loop-safe for reps>1 too: For_i resets semaphores between iterations
    # (reset_sem_bb), so per-iteration absolute wait thresholds and the
    # renumbering stay consistent; all PE-sem updates live in the body.
    _strip_dead_pe_updates(nc)
    _split_waits(nc)
    return nc


_PROGRAM = None


def _get_program():
    global _PROGRAM
    if _PROGRAM is None:
        _PROGRAM = build_program()
    return _PROGRAM


def prepare_in_maps(q, k, v, Wq, bq, Wk, bk, Wv, bv, Wo, bo):
    bf = ml_dtypes.bfloat16
    f32 = np.float32

    def t_bf16(x):  # [B,S,D] f32 -> [B,D,S] bf16 contiguous
        return np.ascontiguousarray(
            np.asarray(x, f32).astype(bf).transpose(0, 2, 1)
        )

    qT = t_bf16(q)
    kT = t_bf16(k)
    vT = t_bf16(v)
    Wq_b = np.asarray(Wq, f32).astype(bf)
    Wk_b = np.asarray(Wk, f32).astype(bf)
    Wv_b = np.asarray(Wv, f32).astype(bf)
    Wo_b = np.asarray(Wo, f32).astype(bf)
    bq2 = np.ascontiguousarray(
        (np.asarray(bq, f32) * np.float32(SCALE)).reshape(KC, 128).T
    )
    bk2 = np.ascontiguousarray(np.asarray(bk, f32).reshape(KC, 128).T)
    bv1 = np.ascontiguousarray(np.asarray(bv, f32)).astype(bf)
    bo1 = np.ascontiguousarray(np.asarray(bo, f32)).astype(bf)

    in_maps = []
    for c in range(N_CORES):
        sl = slice(c * NB, (c + 1) * NB)
        in_maps.append({
            "qT": qT[sl], "kT": kT[sl], "vT": vT[sl],
            "Wq": Wq_b, "Wk": Wk_b, "Wv": Wv_b, "Wo": Wo_b,
            "bq": bq2, "bk": bk2, "bv": bv1, "bo": bo1,
        })
    return in_maps


def kernel(q, k, v, Wq, bq, Wk, bk, Wv, bv, Wo, bo):
    nc = _get_program()
    in_maps = prepare_in_maps(q, k, v, Wq, bq, Wk, bk, Wv, bv, Wo, bo)
    res = run_bass_kernel_spmd(nc, in_maps, core_ids=list(range(N_CORES)))
    out = np.concatenate([res.results[c]["out"] for c in range(N_CORES)], axis=0)
    return out.astype(np.float32)



# revision 17
# speedup vs baseline: 1.4445x; 1.3606x over previous
"""CrossAttention (single-head) Trainium2 kernel, 8-core data-parallel.

Full inputs in, full output out. Internally: batch 16 is sharded 2-per-core
across 8 NeuronCores; each core runs the whole attention layer for its two
batches in bf16 (f32 PSUM accumulation), with activations kept in transposed
[d, s] layout so every matmul contracts over the partition dim without any
on-chip transposes of large tensors.

Projection fusion (exact algebra, softmax-invariant terms dropped):
  scores = (qWq+bq)(kWk+bk)^T/sqrt(D)
         ≡ q (WqWk^T) k^T /sqrt(D) + d[sk]   (+ per-row consts, softmax-inv.)
    with d = k @ (Wk bq)/sqrt(D) folded into the exp activation bias,
  out = attn (vWv+bv) Wo + bo = (attn v)(WvWo) + (bv Wo + bo)
    since attn rows sum to 1.
So the device only runs: one k-projection (with A^T=WkWq^T), the two big
attention matmuls on RAW q/v, and one output projection (with WvWo) —
the q- and v-projections are folded host-side into those matrices.
"""

import sys

sys.path.insert(0, "/opt/trn_rl_repo")

import numpy as np
import ml_dtypes

import concourse.bass as bass
import concourse.mybir as mybir
import concourse.tile as tile
from concourse.bass_utils import run_bass_kernel_spmd

BF16 = mybir.dt.bfloat16
F32 = mybir.dt.float32
AF = mybir.ActivationFunctionType

N_CORES = 8
B, S, D = 16, 2048, 1024
NB = B // N_CORES          # batches per core
KC = D // 128              # 8 chunks of 128 along d
ST = S // 128              # 16 tiles of 128 along s
NBLK = S // 512            # 4 blocks of 512 along s
SCALE = 1.0 / np.sqrt(np.float32(D))  # 1/32


def _split_waits(nc, limit=1):
    """Walrus in this container allows at most one sync wait per instruction:
    hoist excess waits onto NoOp carriers inserted just before."""
    n_new = 0
    for f in nc.m.functions:
        for bb in f.blocks:
            new_insts = []
            for inst in bb.instructions:
                si = inst.sync_info
                waits = list(si.on_wait) if si and si.on_wait else []
                if len(waits) > limit:
                    excess, keep = waits[:-limit], waits[-limit:]
                    for i in range(0, len(excess), limit):
                        chunk = excess[i:i + limit]
                        nop = mybir.InstNoOp(
                            name=f"{inst.name}-ws-{n_new}",
                            ins=[], outs=[],
                            sync_info=mybir.SyncInfo(on_wait=chunk, on_update=[]),
                        )
                        nop.engine = inst.engine
                        new_insts.append(nop)
                        n_new += 1
                    si.on_wait = keep
                new_insts.append(inst)
            bb.instructions[:] = new_insts
    return n_new


def _strip_dead_pe_updates(nc):
    """Drop PE sem increments nobody waits on (Tile emits one per matmul;
    only group-stop indices are ever waited). Renumber wait thresholds by
    rank among kept updates — release timing is identical, PE saves the
    serialized EVT_SEM write per dropped update. Loop-safe: For_i resets
    semaphores between iterations (reset_sem_bb) and all PE-sem updates
    live in the body."""
    pe = mybir.EngineType.PE
    insts = [i for f in nc.m.functions for bb in f.blocks for i in bb.instructions]
    upd_by_sem, wait_by_sem, bad = {}, {}, set()
    for inst in insts:
        si = inst.sync_info
        if not si:
            continue
        for u in (si.on_update or []):
            if u.sync_type != "semaphore":
                continue
            if inst.engine != pe or u.update_mode != "sem-inc" or u.update_value != 1:
                bad.add(u.id)
            upd_by_sem.setdefault(u.id, []).append((inst, u))
        for w in (si.on_wait or []):
            if w.sync_type != "semaphore":
                continue
            if w.wait_mode != "sem-ge-imm" or w.wait_reg is not None:
                bad.add(w.id)
            wait_by_sem.setdefault(w.id, []).append(w)
    n_drop = 0
    for sem_id, ups in upd_by_sem.items():
        if sem_id in bad or sem_id not in wait_by_sem or len(ups) < 16:
            continue
        waited = sorted({w.wait_value for w in wait_by_sem[sem_id]})
        if not waited or waited[-1] > len(ups) or waited[0] < 1:
            continue
        keep = set(waited)
        rank = {t: k + 1 for k, t in enumerate(waited)}
        for idx, (inst, u) in enumerate(ups, start=1):
            if idx not in keep:
                inst.sync_info.on_update = [
                    x for x in inst.sync_info.on_update if x is not u
                ]
                n_drop += 1
        for w in wait_by_sem[sem_id]:
            w.wait_value = rank[w.wait_value]
    return n_drop


def _dedupe_ldweights(nc):
    """Drop InstLdweights whose weights AP equals the previous PE weight
    load with no intervening PE-array clobber (transpose or different
    load): the matmuls are non-self-loading (ldweights=False) so they
    reuse the already-loaded stationary operand. Sync carried by a
    dropped LDW transfers to the next PE instruction."""
    pe = mybir.EngineType.PE
    n_drop = 0
    for f in nc.m.functions:
        for bb in f.blocks:
            insts = bb.instructions
            keep = []
            last_sig = None
            pend_waits, pend_ups = [], []
            for inst in insts:
                tn = type(inst).__name__
                eng = getattr(inst, "engine", None)
                if tn == "InstLdweights":
                    ap = inst.ins[0]
                    sig = (ap.memref, ap.offset, str(ap.ap), str(ap.dtype))
                    if sig == last_sig:
                        si = inst.sync_info
                        if si:
                            pend_waits += list(si.on_wait or [])
                            pend_ups += list(si.on_update or [])
                        n_drop += 1
                        continue
                    last_sig = sig
                elif eng == pe:
                    if tn == "InstMatmult":
                        if inst.is_transpose:
                            last_sig = None
                    elif tn not in ("InstNoOp", "InstEventSemaphore", "InstDrain"):
                        last_sig = None
                if (pend_waits or pend_ups) and eng == pe:
                    si = inst.sync_info
                    if si is None:
                        inst.sync_info = mybir.SyncInfo(
                            on_wait=pend_waits, on_update=pend_ups
                        )
                    else:
                        si.on_wait = pend_waits + list(si.on_wait or [])
                        si.on_update = list(si.on_update or []) + pend_ups
                    pend_waits, pend_ups = [], []
                keep.append(inst)
            assert not pend_waits and not pend_ups, "dangling LDW sync at block end"
            insts[:] = keep
    return n_drop


def build_program(reps=1):
    """reps>1 wraps the whole computation in a hardware For_i loop — used
    only for timing (slope over reps isolates on-silicon exec time from
    per-call NEFF load overhead)."""
    nc = bass.Bass()

    qT_d = nc.declare_dram_parameter("qT", [NB, D, S], BF16, isOutput=False)
    kT_d = nc.declare_dram_parameter("kT", [NB, D, S], BF16, isOutput=False)
    vN_d = nc.declare_dram_parameter("vN", [NB, S, D], BF16, isOutput=False)
    # Wk_in = Wk @ Wq^T (so the k-pass computes (WqWk^T) @ kT)
    Wk_d = nc.declare_dram_parameter("Wk", [D, D], BF16, isOutput=False)
    # Wo_in = Wv @ Wo
    Wo_d = nc.declare_dram_parameter("Wo", [D, D], BF16, isOutput=False)
    # wd[:, i] = chunk i of SCALE*(Wk @ bq) — lhsT chunks for the d matvec
    wd_d = nc.declare_dram_parameter("wd", [128, KC], BF16, isOutput=False)
    # bo_in = bv @ Wo + bo
    bo_d = nc.declare_dram_parameter("bo", [D], BF16, isOutput=False)
    out_d = nc.declare_dram_parameter("out", [NB, S, D], F32, isOutput=True)

    from contextlib import ExitStack
    with tile.TileContext(nc) as tc:
        with ExitStack() as _stk:
            _p = lambda **kw: _stk.enter_context(tc.tile_pool(**kw))
            wopool = _p(name="wo", bufs=8)
            wkpool = _p(name="wk", bufs=9)
            inpool = _p(name="inp", bufs=16)
            kpool = _p(name="keyT", bufs=8)
            vpool = _p(name="value", bufs=1)
            epool = _p(name="expT", bufs=2)
            upool = _p(name="UT", bufs=2)
            opool = _p(name="outb", bufs=2)
            sumpool = _p(name="sums", bufs=2)
            rpool = _p(name="rpool", bufs=2)
            dpool = _p(name="dpool", bufs=2)
            cpool = _p(name="const", bufs=1)
            pspool = _p(name="ps", bufs=5, space="PSUM")
            ps1pool = _p(name="ps1", bufs=1, space="PSUM")
            psrpool = _p(name="psr", bufs=2, space="PSUM")
            # constants
            ones = cpool.tile([128, 1], BF16, tag="ones")
            nc.vector.memset(ones[:], 1.0)
            ident = cpool.tile([1, 1], F32, tag="ident")
            nc.vector.memset(ident[:], 1.0)
            wd_sb = cpool.tile([128, KC], BF16, tag="wd")
            nc.sync.dma_start(out=wd_sb[:], in_=wd_d[:])
            bo_sb = cpool.tile([128, D], BF16, tag="bo")
            ap = bo_d[:]
            nc.sync.dma_start(
                out=bo_sb[:],
                in_=bass.AP(tensor=ap.tensor, offset=ap.offset, ap=[[0, 128]] + ap.ap),
            )

            def load_w(w_d, pool, tag):
                tiles = []
                for i in range(KC):
                    t = pool.tile([128, D], BF16, tag=tag, name=f"{tag}{i}")
                    nc.sync.dma_start(out=t[:], in_=w_d[i * 128:(i + 1) * 128, :])
                    tiles.append(t)
                return tiles

            # critical-path first: Wk and the first kin block feed the very
            # first matmuls — queue them ahead of the resident Wo load.
            # Only for the straight-line (reps==1) program: inside a For_i
            # the hoisted tiles' ring slots get recycled across iterations.
            hoist = reps == 1
            if hoist:
                Wk_first = load_w(Wk_d, wkpool, "wk")
                kin_first = []
                for i in range(KC):
                    t = inpool.tile([128, 512], BF16, tag="inp", name=f"in{i}")
                    nc.sync.dma_start(out=t[:], in_=kT_d[0, i * 128:(i + 1) * 128, 0:512])
                    kin_first.append(t)
            # Wo stays resident for the whole kernel
            Wo_t = load_w(Wo_d, wopool, "wo")

            import contextlib
            loop_ctx = tc.For_i(0, reps, 1) if reps > 1 else contextlib.nullcontext()
            with loop_ctx:
              for b in range(NB):
                  # ------- keyT[d, sk] = (WqWk^T) @ kT;  d[sk] = wd^T @ kT -------
                  Wk_t = Wk_first if (hoist and b == 0) else load_w(Wk_d, wkpool, "wk")
                  keyT = [kpool.tile([128, S], BF16, tag="keyT", name=f"keyT{i}") for i in range(KC)]
                  d_sb = dpool.tile([1, S], F32, tag="dsb")
                  for s in range(NBLK):
                      if hoist and b == 0 and s == 0:
                          kin = kin_first
                      else:
                          kin = []
                          for i in range(KC):
                              t = inpool.tile([128, 512], BF16, tag="inp", name=f"in{i}")
                              nc.sync.dma_start(
                                  out=t[:],
                                  in_=kT_d[b, i * 128:(i + 1) * 128, s * 512:(s + 1) * 512],
                              )
                              kin.append(t)
                      for do in range(KC):
                          psum = pspool.tile([128, 512], F32, tag="ps")
                          for i in range(KC):
                              nc.tensor.matmul(
                                  psum[:], Wk_t[i][:, do * 128:(do + 1) * 128], kin[i][:],
                                  start=(i == 0), stop=(i == KC - 1),
                              )
                          nc.vector.tensor_copy(
                              keyT[do][:, s * 512:(s + 1) * 512], psum[:],
                          )
                      # d matvec rides on the resident kin tiles
                      ps_d = ps1pool.tile([1, 512], F32, tag="ps1")
                      for i in range(KC):
                          nc.tensor.matmul(
                              ps_d[:], wd_sb[:, i:i + 1], kin[i][:],
                              start=(i == 0), stop=(i == KC - 1),
                          )
                      nc.vector.tensor_copy(d_sb[0:1, s * 512:(s + 1) * 512], ps_d[:])

                  # d as per-partition scalars [128, ST] via PE transposes
                  d_part = dpool.tile([128, ST], F32, tag="dpart")
                  for t16 in range(ST):
                      pr = psrpool.tile([128, 1], F32, tag="psr")
                      nc.tensor.transpose(
                          pr[:], d_sb[0:1, t16 * 128:(t16 + 1) * 128], ident[:]
                      )
                      nc.vector.tensor_copy(d_part[:, t16:t16 + 1], pr[:])

                  # ------- value[s, d]: raw v, direct DMA (v-proj folded) -------
                  val = vpool.tile([128, ST, D], BF16, tag="value")
                  for t16 in range(ST):
                      nc.sync.dma_start(
                          out=val[:, t16, :],
                          in_=vN_d[b, t16 * 128:(t16 + 1) * 128, :],
                      )

                  # ---------------- per 512-wide sq block ----------------
                  for blk in range(NBLK):
                      # raw qT block feeds scores directly (q-proj folded)
                      qin = []
                      for i in range(KC):
                          t = inpool.tile([128, 512], BF16, tag="inp", name=f"in{i}")
                          nc.sync.dma_start(
                              out=t[:],
                              in_=qT_d[b, i * 128:(i + 1) * 128, blk * 512:(blk + 1) * 512],
                          )
                          qin.append(t)

                      # scoresT -> expT;  exp(psum*SCALE + d[sk])
                      exp_blk = epool.tile([128, ST, 512], BF16, tag="expT")
                      for t16 in range(ST):
                          psum = pspool.tile([128, 512], F32, tag="ps")
                          for i in range(KC):
                              nc.tensor.matmul(
                                  psum[:],
                                  keyT[i][:, t16 * 128:(t16 + 1) * 128],
                                  qin[i][:],
                                  start=(i == 0), stop=(i == KC - 1),
                              )
                          nc.scalar.activation(
                              exp_blk[:, t16, :], psum[:], AF.Exp,
                              bias=d_part[:, t16:t16 + 1], scale=float(SCALE),
                          )

                      # column sums over all sk (partition dim) via ones-matmul
                      sums_ps = ps1pool.tile([1, 512], F32, tag="ps1")
                      for t16 in range(ST):
                          nc.tensor.matmul(
                              sums_ps[:], ones[:], exp_blk[:, t16, :],
                              start=(t16 == 0), stop=(t16 == ST - 1),
                          )
                      sums_sb = sumpool.tile([1, 512], F32, tag="sums")
                      nc.vector.tensor_copy(sums_sb[:], sums_ps[:])

                      # r = 1/sums as per-partition scalars, via [1,128] PE
                      # transpose; emitted before UT so its PE<->DVE chain is
                      # hidden under the UT matmul stream
                      r_sb = rpool.tile([128, 4], F32, tag="r")
                      for m in range(4):
                          pr = psrpool.tile([128, 1], F32, tag="psr")
                          nc.tensor.transpose(
                              pr[:], sums_sb[0:1, m * 128:(m + 1) * 128], ident[:]
                          )
                          nc.vector.reciprocal(r_sb[:, m:m + 1], pr[:])

                      # UT block [d, 512] = val.T @ expT  (raw v)
                      ut = upool.tile([128, KC, 512], BF16, tag="UT")
                      for j in range(KC):
                          psum = pspool.tile([128, 512], F32, tag="ps")
                          for t16 in range(ST):
                              nc.tensor.matmul(
                                  psum[:],
                                  val[:, t16, j * 128:(j + 1) * 128],
                                  exp_blk[:, t16, :],
                                  start=(t16 == 0), stop=(t16 == ST - 1),
                              )
                          nc.vector.tensor_copy(ut[:, j, :], psum[:])

                      # final block: out[sq, d] = (UT.T @ (WvWo)) * r + bo_in
                      for m in range(4):
                          ob = opool.tile([128, D], F32, tag="outb")
                          for n in range(2):
                              psum = pspool.tile([128, 512], F32, tag="ps")
                              for j in range(KC):
                                  nc.tensor.matmul(
                                      psum[:],
                                      ut[:, j, m * 128:(m + 1) * 128],
                                      Wo_t[j][:, n * 512:(n + 1) * 512],
                                      start=(j == 0), stop=(j == KC - 1),
                                  )
                              nc.vector.tensor_scalar_mul(
                                  ob[:, n * 512:(n + 1) * 512], psum[:], r_sb[:, m:m + 1]
                              )
                              nc.vector.tensor_add(
                                  ob[:, n * 512:(n + 1) * 512],
                                  ob[:, n * 512:(n + 1) * 512],
                                  bo_sb[:, n * 512:(n + 1) * 512],
                              )
                          sq = blk * 512 + m * 128
                          nc.sync.dma_start(out=out_d[b, sq:sq + 128, :], in_=ob[:])

    _dedupe_ldweights(nc)
    # Loop-safe for reps>1 too: For_i resets semaphores between iterations.
    _strip_dead_pe_updates(nc)
    _split_waits(nc)
    return nc


_PROGRAM = None


def _get_program():
    global _PROGRAM
    if _PROGRAM is None:
        _PROGRAM = build_program()
    return _PROGRAM


def prepare_in_maps(q, k, v, Wq, bq, Wk, bk, Wv, bv, Wo, bo):
    bf = ml_dtypes.bfloat16
    f32 = np.float32

    def t_bf16(x):  # [B,S,D] f32 -> [B,D,S] bf16 contiguous
        return np.ascontiguousarray(
            np.asarray(x, f32).astype(bf).transpose(0, 2, 1)
        )

    qT = t_bf16(q)
    kT = t_bf16(k)
    vN = np.ascontiguousarray(np.asarray(v, f32).astype(bf))

    Wq_f = np.asarray(Wq, f32); Wk_f = np.asarray(Wk, f32)
    Wv_f = np.asarray(Wv, f32); Wo_f = np.asarray(Wo, f32)
    bq_f = np.asarray(bq, f32); bv_f = np.asarray(bv, f32)
    bo_f = np.asarray(bo, f32)

    Wk_in = np.ascontiguousarray(Wk_f @ Wq_f.T).astype(bf)      # (WqWk^T)^T
    Wo_in = np.ascontiguousarray(Wv_f @ Wo_f).astype(bf)        # WvWo
    wd = np.ascontiguousarray(
        ((Wk_f @ bq_f) * np.float32(SCALE)).reshape(KC, 128).T
    ).astype(bf)
    bo_in = np.ascontiguousarray(bv_f @ Wo_f + bo_f).astype(bf)

    in_maps = []
    for c in range(N_CORES):
        sl = slice(c * NB, (c + 1) * NB)
        in_maps.append({
            "qT": qT[sl], "kT": kT[sl], "vN": vN[sl],
            "Wk": Wk_in, "Wo": Wo_in, "wd": wd, "bo": bo_in,
        })
    return in_maps


def kernel(q, k, v, Wq, bq, Wk, bk, Wv, bv, Wo, bo):
    nc = _get_program()
    in_maps = prepare_in_maps(q, k, v, Wq, bq, Wk, bk, Wv, bv, Wo, bo)
    res = run_bass_kernel_spmd(nc, in_maps, core_ids=list(range(N_CORES)))
    out = np.concatenate([res.results[c]["out"] for c in range(N_CORES)], axis=0)
    return out.astype(np.float32)
